# revision 21
# baseline (speedup 1.0000x reference)
"""Trainium2 kernel for the algo/task performance-scan problem.

The lax.scan's only cross-step dependency is the 64 scalars sig[:, lx[l]]
read each step.  That scalar chain (O(A*L + L^2)) runs on the host in
float64.  Given per-step coefficients c[a,l] = eff[a] + s[a,l]*boost[a],
the field is a banded matmul

    result[a, l, t] = sum_{j<=l} mem[a]^(l-j) * c[a,j] * row_j[t]

followed by sig = tanh(result / (2*diff))  (2*sigmoid(x)-1 = tanh(x/2)).

Device design (single precision pass, tasks sorted by difficulty):
  * 1/(2*diff[t]) folds into R' = task_matrix[lx]/(2 diff), so PSUM holds
    x = result/(2 diff) directly.
  * Tasks are SORTED by difficulty.  The three lowest-difficulty task
    blocks (slope up to 10) use f16 operands (2 cycles/row on HW but
    11-bit mantissa); the five high blocks use bf16 (1 cycle/row).
    Measured per-block max err stays under 1.1e-2 (gate 2e-2).
  * All output is int8 (1 B/elem DMA):
      - blocks 0-5: ACT tanh (PSUM f32 -> SBUF f16), DVE *126.5 -> int8.
      - blocks 6-7: x itself is quantized (int8 = x * s_t with a rigorous
        host-side bound s_t = 127/Bx_t); host computes tanh.  One ts op,
        no ACT tanh -- relieves the ACT engine.
  * R' is stored as 7 overlapping 128-row j-chunks per dtype so every
    l-tile is one K=128 matmul (K=64 partition-offset matmuls fault).
"""

import sys

sys.path.insert(0, "/opt/trn_rl_repo")

import numpy as np

A, T, L = 64, 1024, 512
NCORES = 8
ACORE = A // NCORES          # 8 algos per core
LT = 64                      # l-tile size
NLT = L // LT                # 8 l-tiles
NTB = T // 128               # 8 task blocks (sorted by difficulty)
NG = 2                       # output groups (4 l-tiles each)
NF = 3                       # t-blocks 0..NF-1 use f16 matmul operands
NX = 6                       # t-blocks >= NX use int8-x output (no tanh)
TF = NF * 128                # f16 task columns
TB = T - TF                  # bf16 task columns

_CACHE = {}


def _build_program():
    import concourse.tile as tile
    from concourse import bacc, mybir

    nc = bacc.Bacc("TRN2", target_bir_lowering=False, debug=False,
                   enable_asserts=False, num_devices=NCORES)
    f32 = mybir.dt.float32
    f16 = mybir.dt.float16
    bf16 = mybir.dt.bfloat16
    i8 = mybir.dt.int8

    rpf0_in = nc.dram_tensor("rpf0", [3, 128, TF], f16,
                             kind="ExternalInput").ap()
    rpf1_in = nc.dram_tensor("rpf1", [4, 128, TF], f16,
                             kind="ExternalInput").ap()
    rpb0_in = nc.dram_tensor("rpb0", [3, 128, TB], bf16,
                             kind="ExternalInput").ap()
    rpb1_in = nc.dram_tensor("rpb1", [4, 128, TB], bf16,
                             kind="ExternalInput").ap()
    gf0_in = nc.dram_tensor("gf0", [4, 128, ACORE * LT], f16,
                            kind="ExternalInput").ap()
    gf1_in = nc.dram_tensor("gf1", [4, 128, ACORE * LT], f16,
                            kind="ExternalInput").ap()
    gb0_in = nc.dram_tensor("gb0", [4, 128, ACORE * LT], bf16,
                            kind="ExternalInput").ap()
    gb1_in = nc.dram_tensor("gb1", [4, 128, ACORE * LT], bf16,
                            kind="ExternalInput").ap()
    dsc_in = nc.dram_tensor("dsc", [128, NTB - NX], f32,
                            kind="ExternalInput").ap()
    out8 = nc.dram_tensor("out8", [NG * NTB, 128, ACORE * 256], i8,
                          kind="ExternalOutput").ap()

    with tile.TileContext(nc) as tc:
        with tc.tile_pool(name="consts", bufs=1) as consts, \
             tc.tile_pool(name="stage", bufs=4) as stage, \
             tc.tile_pool(name="stage8", bufs=4) as stage8, \
             tc.tile_pool(name="ps", bufs=2, space="PSUM") as psp:

            # Pre-load the tanh ACT table during the input-DMA lead-in.
            wsrc = consts.tile([128, 64], f16, tag="warm")
            wdst = consts.tile([128, 64], f16, tag="warmout")
            nc.gpsimd.memset(wsrc[:], 0.0)
            nc.scalar.activation(wdst[:], wsrc[:],
                                 mybir.ActivationFunctionType.Tanh,
                                 scale=1.0)

            def bulk(tag, src, n, width, dt):
                t_ = consts.tile([128, n * width], dt, tag=tag)

                def issue():
                    nc.sync.dma_start(
                        t_[:].rearrange("p (c w) -> p c w", c=n),
                        src.rearrange("c p w -> p c w"))
                return t_, issue

            # Allocate all input tiles; issue only the first six tiles'
            # inputs up front.  The rest are issued later in Sync program
            # order (gated behind early out-DMA waits) so the first tile's
            # operands aren't stuck behind 3.75MB of round-robin DMA.
            rpf0, i_rpf0 = bulk("rpf0", rpf0_in, 3, TF, f16)
            gf0, i_gf0 = bulk("gf0", gf0_in, 4, ACORE * LT, f16)
            rpb0, i_rpb0 = bulk("rpb0", rpb0_in, 3, TB, bf16)
            gb0, i_gb0 = bulk("gb0", gb0_in, 4, ACORE * LT, bf16)
            rpf1, i_rpf1 = bulk("rpf1", rpf1_in, 4, TF, f16)
            gf1, i_gf1 = bulk("gf1", gf1_in, 4, ACORE * LT, f16)
            rpb1, i_rpb1 = bulk("rpb1", rpb1_in, 4, TB, bf16)
            gb1, i_gb1 = bulk("gb1", gb1_in, 4, ACORE * LT, bf16)
            dsc = consts.tile([128, NTB - NX], f32, tag="dsc")
            i_rpf0(); i_gf0(); i_rpb0(); i_gb0()
            nc.sync.dma_start(dsc[:], dsc_in)
            i_rpf1(); i_gf1(); i_rpb1(); i_gb1()

            # R' chunk for each l-tile: chunk windows at j0 =
            # [0, 0, 64, 128, 192, 256, 320, 384] for lt 0..7
            lt_chunk = [(0, 0), (0, 0), (0, 1), (0, 2),
                        (1, 0), (1, 1), (1, 2), (1, 3)]

            def rchunk(lt, use_f16):
                half, i = lt_chunk[lt]
                if use_f16:
                    rt, w = ((rpf0, rpf1)[half], TF)
                else:
                    rt, w = ((rpb0, rpb1)[half], TB)
                return rt[:, i * w:(i + 1) * w]

            W = ACORE * LT

            def gslice(lt, use_f16):
                gt = ((gf0, gf1) if use_f16 else (gb0, gb1))[lt // 4]
                return gt[:, (lt % 4) * W:(lt % 4 + 1) * W]

            # int8-x tiles (tb 6,7) interleaved between tanh tiles so the
            # ACT chain never serializes the pipeline; g1 ends on an x-tile
            # for a short tail.  GpSimd tensor_scalar measured ~29us/tile
            # on HW (~25x the cost-model estimate) -- never use it.
            TB_ORDER = [[0, 1, 2, 3, 6, 4, 7, 5],
                        [0, 1, 2, 3, 6, 4, 5, 7]]
            for g in range(NG):
                for pos, tb in enumerate(TB_ORDER[g]):
                    use_f16 = tb < NF
                    tcol = tb * 128 if use_f16 else (tb - NF) * 128
                    ps = psp.tile([128, 4 * W], f32, tag="ps")
                    for sub in range(4):
                        lt = g * 4 + sub
                        psl = ps[:, sub * W:(sub + 1) * W]
                        rt = rchunk(lt, use_f16)
                        nc.tensor.matmul(
                            psl, lhsT=rt[:, tcol:tcol + 128],
                            rhs=gslice(lt, use_f16), start=True, stop=True)
                    idx = g * 8 + tb
                    last = g == NG - 1 and pos == NTB - 1
                    # psum free layout: s*W + a*64 + ll
                    # sbuf free layout: a*256 + s*64 + ll (contiguous runs)
                    ps_r = ps[:].rearrange("p (s a l) -> p s a l", s=4,
                                           a=ACORE)
                    ob = stage8.tile([128, ACORE * 256], i8, tag="ob")
                    if tb < NX:
                        th = stage.tile([128, ACORE * 256], f16, tag="th")
                        nc.scalar.activation(
                            th[:].rearrange("p (a s l) -> p s a l",
                                            a=ACORE, s=4),
                            ps_r, mybir.ActivationFunctionType.Tanh,
                            scale=1.0)
                        nc.vector.tensor_scalar(
                            ob[:], th[:], 126.5, None, mybir.AluOpType.mult)
                        nc.sync.dma_start(out8[idx], ob[:])
                    else:
                        scol = dsc[:, tb - NX:tb - NX + 1]
                        ob_r = ob[:].rearrange("p (a s l) -> p s a l",
                                               a=ACORE, s=4)
                        # final tile: drain+store in halves so the last DMA
                        # overlaps the last tensor_scalar
                        halves = [(0, 2), (2, 4)] if last else [(0, 4)]
                        for h0, h1 in halves:
                            nc.vector.tensor_scalar(
                                ob_r[:, h0:h1], ps_r[:, h0:h1], scol, None,
                                mybir.AluOpType.mult)
                            nc.sync.dma_start(
                                out8[idx].rearrange(
                                    "p (a sl) -> p a sl", a=ACORE)
                                [:, :, h0 * 64:h1 * 64],
                                ob[:].rearrange("p (a sl) -> p a sl",
                                                a=ACORE)
                                [:, :, h0 * 64:h1 * 64])

    nc.compile()
    return nc


def _host_chain(lx, task_matrix, task_difficulty, alg_efficiency,
                alg_memory, alg_experience_boost):
    """Exact (f64) scalar feedback chain; returns per-core input maps."""
    import ml_dtypes
    bf = ml_dtypes.bfloat16

    lx = np.asarray(lx).astype(np.int64)
    TM = np.asarray(task_matrix, dtype=np.float64)
    diff = np.asarray(task_difficulty, dtype=np.float64)
    eff = np.asarray(alg_efficiency, dtype=np.float64)
    mem = np.asarray(alg_memory, dtype=np.float64)
    boost = np.asarray(alg_experience_boost, dtype=np.float64)

    R = TM[lx]                     # [L, T]
    TM2 = R[:, lx]                 # [L, L]
    dlx = diff[lx]                 # [L]

    resS = np.zeros((A, L))
    c = np.empty((A, L))
    for l in range(L):
        s_l = 2.0 / (1.0 + np.exp(-resS[:, l] / dlx[l])) - 1.0
        c[:, l] = eff + s_l * boost
        resS = resS * mem[:, None] + c[:, l][:, None] * TM2[l][None, :]

    order = np.argsort(diff, kind="stable")
    dsort = diff[order]
    Rs = R[:, order]
    Rp = Rs / (2.0 * dsort[None, :])     # [L, T] sorted tasks

    # rigorous per-task bound on |x| for the int8-x blocks
    cmax = c.max()
    memmax = mem.max()
    b = np.zeros(T)
    bmax = np.zeros(T)
    for l in range(L):
        b = memmax * b + cmax * np.abs(Rp[l])
        bmax = np.maximum(bmax, b)
    s_t = 127.0 / np.maximum(bmax, 1e-6)  # int8 = round(x * s_t)
    dsc = np.ascontiguousarray(
        s_t[TF + (NX - NF) * 128:].reshape(NTB - NX, 128).T).astype(np.float32)

    # G[a, lt, jj, ll] = mem^(l-j) * c[a, j], j = js(lt)+jj, l = 64*lt+ll
    pmat = mem[:, None] ** np.arange(192)[None, :]        # [A, 192]
    G = np.zeros((A, NLT, 128, LT))
    for lt in range(NLT):
        js = 0 if lt == 0 else 64 * (lt - 1)
        jw = np.arange(js, js + 128)
        lmj = (np.arange(LT)[None, :] + 64 * lt) - jw[:, None]   # [128, LT]
        valid = lmj >= 0
        G[:, lt] = np.where(valid[None],
                            pmat[:, np.maximum(lmj, 0)] * c[:, jw][:, :, None],
                            0.0)

    starts0, starts1 = (0, 64, 128), (192, 256, 320, 384)

    def rpack(Rx, starts):
        return np.ascontiguousarray(np.stack([Rx[s:s + 128] for s in starts]))

    Rpf = Rp[:, :TF].astype(np.float16)
    Rpb = Rp[:, TF:].astype(bf)
    rp = {"rpf0": rpack(Rpf, starts0), "rpf1": rpack(Rpf, starts1),
          "rpb0": rpack(Rpb, starts0), "rpb1": rpack(Rpb, starts1),
          "dsc": dsc}

    in_maps = []
    for core in range(NCORES):
        blk = G[core * ACORE:(core + 1) * ACORE]     # [ACORE, NLT, 128, LT]
        gp = np.ascontiguousarray(
            blk.transpose(1, 2, 0, 3).reshape(NLT, 128, ACORE * LT))
        gpf = gp.astype(np.float16)
        gpb = gp.astype(bf)
        in_maps.append({
            **rp,
            "gf0": np.ascontiguousarray(gpf[:4]),
            "gf1": np.ascontiguousarray(gpf[4:]),
            "gb0": np.ascontiguousarray(gpb[:4]),
            "gb1": np.ascontiguousarray(gpb[4:]),
        })
    return in_maps, order, s_t


def kernel(lx, task_matrix, task_difficulty, alg_efficiency, alg_memory,
           alg_experience_boost):
    from concourse.bass_utils import run_bass_kernel_spmd

    in_maps, order, s_t = _host_chain(
        lx, task_matrix, task_difficulty, alg_efficiency, alg_memory,
        alg_experience_boost)

    if "nc" not in _CACHE:
        _CACHE["nc"] = _build_program()
    nc = _CACHE["nc"]

    res = run_bass_kernel_spmd(nc, in_maps, core_ids=list(range(NCORES)),
                               trace=False)
    srt = np.empty((A, T, L), dtype=np.float32)   # sorted-task sig field
    for cidx in range(NCORES):
        d8 = res.results[cidx]["out8"]            # [16, 128, 2048] int8
        for idx in range(NG * NTB):
            g, tb = idx // 8, idx % 8
            arr = d8[idx].astype(np.float32)      # [128t, (a,s,ll)]
            if tb < NX:
                sig = arr / 126.5
            else:
                x = arr / s_t[tb * 128:(tb + 1) * 128][:, None]
                sig = np.tanh(x)
            sig = sig.reshape(128, ACORE, 256).transpose(1, 0, 2)
            srt[cidx * ACORE:(cidx + 1) * ACORE,
                tb * 128:(tb + 1) * 128,
                g * 256:(g + 1) * 256] = sig
    out = np.empty((A, T, L + 1), dtype=np.float32)
    out[:, :, 0] = 0.0
    out[:, order, 1:] = srt
    return out


# revision 23
# speedup vs baseline: 1.1885x; 1.1885x over previous
"""Trainium2 kernel for the algo/task performance-scan problem.

The lax.scan's only cross-step dependency is the 64 scalars sig[:, lx[l]]
read each step.  That scalar chain (O(A*L + L^2)) runs on the host in
float64.  Given per-step coefficients c[a,l] = eff[a] + s[a,l]*boost[a],
the field is a banded matmul

    result[a, l, t] = sum_{j<=l} mem[a]^(l-j) * c[a,j] * row_j[t]

followed by sig = tanh(result / (2*diff))  (2*sigmoid(x)-1 = tanh(x/2)).

Device design (single precision pass, tasks sorted by difficulty):
  * 1/(2*diff[t]) folds into R' = task_matrix[lx]/(2 diff), so PSUM holds
    x = result/(2 diff) directly.
  * Tasks are SORTED by difficulty.  The three lowest-difficulty task
    blocks (slope up to 10) use f16 operands (2 cycles/row on HW but
    11-bit mantissa); the five high blocks use bf16 (1 cycle/row).
    Measured per-block max err stays under 1.1e-2 (gate 2e-2).
  * All output is int8 (1 B/elem DMA):
      - blocks 0-5: ACT tanh (PSUM f32 -> SBUF f16), DVE *126.5 -> int8.
      - blocks 6-7: x itself is quantized (int8 = x * s_t with a rigorous
        host-side bound s_t = 127/Bx_t); host computes tanh.  One ts op,
        no ACT tanh -- relieves the ACT engine.
  * R' is stored as 7 overlapping 128-row j-chunks per dtype so every
    l-tile is one K=128 matmul (K=64 partition-offset matmuls fault).
"""

import sys

sys.path.insert(0, "/opt/trn_rl_repo")

import numpy as np

A, T, L = 64, 1024, 512
NCORES = 8
ACORE = A // NCORES          # 8 algos per core
LT = 64                      # l-tile size
NLT = L // LT                # 8 l-tiles
NTB = T // 128               # 8 task blocks (sorted by difficulty)
NG = 2                       # output groups (4 l-tiles each)
NF = 3                       # t-blocks 0..NF-1 use f16 matmul operands
NX = 6                       # t-blocks >= NX use int8-x output (no tanh)
TF = NF * 128                # f16 task columns
TB = T - TF                  # bf16 task columns

_CACHE = {}


def _build_program():
    import concourse.tile as tile
    from concourse import bacc, mybir

    nc = bacc.Bacc("TRN2", target_bir_lowering=False, debug=False,
                   enable_asserts=False, num_devices=NCORES)
    f32 = mybir.dt.float32
    f16 = mybir.dt.float16
    bf16 = mybir.dt.bfloat16
    i8 = mybir.dt.int8

    rpf0_in = nc.dram_tensor("rpf0", [3, 128, TF], f16,
                             kind="ExternalInput").ap()
    rpf1_in = nc.dram_tensor("rpf1", [4, 128, TF], f16,
                             kind="ExternalInput").ap()
    rpb0_in = nc.dram_tensor("rpb0", [3, 128, TB], bf16,
                             kind="ExternalInput").ap()
    rpb1_in = nc.dram_tensor("rpb1", [4, 128, TB], bf16,
                             kind="ExternalInput").ap()
    gf0_in = nc.dram_tensor("gf0", [4, 128, ACORE * LT], f16,
                            kind="ExternalInput").ap()
    gf1_in = nc.dram_tensor("gf1", [4, 128, ACORE * LT], f16,
                            kind="ExternalInput").ap()
    gb0_in = nc.dram_tensor("gb0", [4, 128, ACORE * LT], bf16,
                            kind="ExternalInput").ap()
    gb1_in = nc.dram_tensor("gb1", [4, 128, ACORE * LT], bf16,
                            kind="ExternalInput").ap()
    dsc_in = nc.dram_tensor("dsc", [128, NTB - NX], f32,
                            kind="ExternalInput").ap()
    out8 = nc.dram_tensor("out8", [NG * NTB, 128, ACORE * 256], i8,
                          kind="ExternalOutput").ap()

    with tile.TileContext(nc) as tc:
        with tc.tile_pool(name="consts", bufs=1) as consts, \
             tc.tile_pool(name="stage", bufs=4) as stage, \
             tc.tile_pool(name="stage8", bufs=4) as stage8, \
             tc.tile_pool(name="ps", bufs=2, space="PSUM") as psp:

            # Pre-load the tanh ACT table during the input-DMA lead-in.
            wsrc = consts.tile([128, 64], f16, tag="warm")
            wdst = consts.tile([128, 64], f16, tag="warmout")
            nc.gpsimd.memset(wsrc[:], 0.0)
            nc.scalar.activation(wdst[:], wsrc[:],
                                 mybir.ActivationFunctionType.Tanh,
                                 scale=1.0)

            def bulk(tag, src, n, width, dt):
                t_ = consts.tile([128, n * width], dt, tag=tag)

                def issue():
                    nc.sync.dma_start(
                        t_[:].rearrange("p (c w) -> p c w", c=n),
                        src.rearrange("c p w -> p c w"))
                return t_, issue

            # Allocate all input tiles; issue only the first six tiles'
            # inputs up front.  The rest are issued later in Sync program
            # order (gated behind early out-DMA waits) so the first tile's
            # operands aren't stuck behind 3.75MB of round-robin DMA.
            rpf0, i_rpf0 = bulk("rpf0", rpf0_in, 3, TF, f16)
            gf0, i_gf0 = bulk("gf0", gf0_in, 4, ACORE * LT, f16)
            rpb0, i_rpb0 = bulk("rpb0", rpb0_in, 3, TB, bf16)
            gb0, i_gb0 = bulk("gb0", gb0_in, 4, ACORE * LT, bf16)
            rpf1, i_rpf1 = bulk("rpf1", rpf1_in, 4, TF, f16)
            gf1, i_gf1 = bulk("gf1", gf1_in, 4, ACORE * LT, f16)
            rpb1, i_rpb1 = bulk("rpb1", rpb1_in, 4, TB, bf16)
            gb1, i_gb1 = bulk("gb1", gb1_in, 4, ACORE * LT, bf16)
            dsc = consts.tile([128, NTB - NX], f32, tag="dsc")
            i_rpf0(); i_gf0(); i_rpb0(); i_gb0()
            nc.sync.dma_start(dsc[:], dsc_in)
            i_rpf1(); i_gf1(); i_rpb1(); i_gb1()

            # R' chunk for each l-tile: chunk windows at j0 =
            # [0, 0, 64, 128, 192, 256, 320, 384] for lt 0..7
            lt_chunk = [(0, 0), (0, 0), (0, 1), (0, 2),
                        (1, 0), (1, 1), (1, 2), (1, 3)]

            def rchunk(lt, use_f16):
                half, i = lt_chunk[lt]
                if use_f16:
                    rt, w = ((rpf0, rpf1)[half], TF)
                else:
                    rt, w = ((rpb0, rpb1)[half], TB)
                return rt[:, i * w:(i + 1) * w]

            W = ACORE * LT

            def gslice(lt, use_f16):
                gt = ((gf0, gf1) if use_f16 else (gb0, gb1))[lt // 4]
                return gt[:, (lt % 4) * W:(lt % 4 + 1) * W]

            # int8-x tiles (tb 6,7) interleaved between tanh tiles so the
            # ACT chain never serializes the pipeline; g1 ends on an x-tile
            # for a short tail.  GpSimd tensor_scalar measured ~29us/tile
            # on HW (~25x the cost-model estimate) -- never use it.
            TB_ORDER = [list(range(NTB)), list(range(NTB))]
            n_x = 0
            for g in range(NG):
                for pos, tb in enumerate(TB_ORDER[g]):
                    use_f16 = tb < NF
                    tcol = tb * 128 if use_f16 else (tb - NF) * 128
                    ps = psp.tile([128, 4 * W], f32, tag="ps")
                    for sub in range(4):
                        lt = g * 4 + sub
                        psl = ps[:, sub * W:(sub + 1) * W]
                        rt = rchunk(lt, use_f16)
                        nc.tensor.matmul(
                            psl, lhsT=rt[:, tcol:tcol + 128],
                            rhs=gslice(lt, use_f16), start=True, stop=True)
                    idx = g * 8 + tb
                    last = g == NG - 1 and pos == NTB - 1
                    # psum free layout: s*W + a*64 + ll
                    # sbuf free layout: a*256 + s*64 + ll (contiguous runs)
                    ps_r = ps[:].rearrange("p (s a l) -> p s a l", s=4,
                                           a=ACORE)
                    ob = stage8.tile([128, ACORE * 256], i8, tag="ob")
                    if tb < NX:
                        th = stage.tile([128, ACORE * 256], f16, tag="th")
                        nc.scalar.activation(
                            th[:].rearrange("p (a s l) -> p s a l",
                                            a=ACORE, s=4),
                            ps_r, mybir.ActivationFunctionType.Tanh,
                            scale=1.0)
                        nc.vector.tensor_scalar(
                            ob[:], th[:], 126.5, None, mybir.AluOpType.mult)
                        nc.sync.dma_start(out8[idx], ob[:])
                    else:
                        scol = dsc[:, tb - NX:tb - NX + 1]
                        ob_r = ob[:].rearrange("p (a s l) -> p s a l",
                                               a=ACORE, s=4)
                        if last:
                            # final tile: drain+store in halves on DVE so
                            # the last DMA overlaps the last tensor_scalar
                            for h0, h1 in ((0, 2), (2, 4)):
                                nc.vector.tensor_scalar(
                                    ob_r[:, h0:h1], ps_r[:, h0:h1], scol,
                                    None, mybir.AluOpType.mult)
                                nc.sync.dma_start(
                                    out8[idx].rearrange(
                                        "p (a sl) -> p a sl", a=ACORE)
                                    [:, :, h0 * 64:h1 * 64],
                                    ob[:].rearrange("p (a sl) -> p a sl",
                                                    a=ACORE)
                                    [:, :, h0 * 64:h1 * 64])
                        else:
                            if n_x % 2 == 0:
                                nc.vector.tensor_scalar(
                                    ob_r, ps_r, scol, None,
                                    mybir.AluOpType.mult)
                            else:
                                nc.scalar.activation(
                                    ob_r, ps_r,
                                    mybir.ActivationFunctionType.Copy,
                                    bias=0.0, scale=scol)
                            n_x += 1
                            nc.sync.dma_start(out8[idx], ob[:])

    nc.compile()
    return nc


def _host_chain(lx, task_matrix, task_difficulty, alg_efficiency,
                alg_memory, alg_experience_boost):
    """Exact (f64) scalar feedback chain; returns per-core input maps."""
    import ml_dtypes
    bf = ml_dtypes.bfloat16

    lx = np.asarray(lx).astype(np.int64)
    TM = np.asarray(task_matrix, dtype=np.float64)
    diff = np.asarray(task_difficulty, dtype=np.float64)
    eff = np.asarray(alg_efficiency, dtype=np.float64)
    mem = np.asarray(alg_memory, dtype=np.float64)
    boost = np.asarray(alg_experience_boost, dtype=np.float64)

    R = TM[lx]                     # [L, T]
    TM2 = R[:, lx]                 # [L, L]
    dlx = diff[lx]                 # [L]

    resS = np.zeros((A, L))
    c = np.empty((A, L))
    for l in range(L):
        s_l = 2.0 / (1.0 + np.exp(-resS[:, l] / dlx[l])) - 1.0
        c[:, l] = eff + s_l * boost
        resS = resS * mem[:, None] + c[:, l][:, None] * TM2[l][None, :]

    order = np.argsort(diff, kind="stable")
    dsort = diff[order]
    Rs = R[:, order]
    Rp = Rs / (2.0 * dsort[None, :])     # [L, T] sorted tasks

    # rigorous per-task bound on |x| for the int8-x blocks
    cmax = c.max()
    memmax = mem.max()
    b = np.zeros(T)
    bmax = np.zeros(T)
    for l in range(L):
        b = memmax * b + cmax * np.abs(Rp[l])
        bmax = np.maximum(bmax, b)
    s_t = 127.0 / np.maximum(bmax, 1e-6)  # int8 = round(x * s_t)
    dsc = np.ascontiguousarray(
        s_t[TF + (NX - NF) * 128:].reshape(NTB - NX, 128).T).astype(np.float32)

    # G[a, lt, jj, ll] = mem^(l-j) * c[a, j], j = js(lt)+jj, l = 64*lt+ll
    pmat = mem[:, None] ** np.arange(192)[None, :]        # [A, 192]
    G = np.zeros((A, NLT, 128, LT))
    for lt in range(NLT):
        js = 0 if lt == 0 else 64 * (lt - 1)
        jw = np.arange(js, js + 128)
        lmj = (np.arange(LT)[None, :] + 64 * lt) - jw[:, None]   # [128, LT]
        valid = lmj >= 0
        G[:, lt] = np.where(valid[None],
                            pmat[:, np.maximum(lmj, 0)] * c[:, jw][:, :, None],
                            0.0)

    starts0, starts1 = (0, 64, 128), (192, 256, 320, 384)

    def rpack(Rx, starts):
        return np.ascontiguousarray(np.stack([Rx[s:s + 128] for s in starts]))

    Rpf = Rp[:, :TF].astype(np.float16)
    Rpb = Rp[:, TF:].astype(bf)
    rp = {"rpf0": rpack(Rpf, starts0), "rpf1": rpack(Rpf, starts1),
          "rpb0": rpack(Rpb, starts0), "rpb1": rpack(Rpb, starts1),
          "dsc": dsc}

    in_maps = []
    for core in range(NCORES):
        blk = G[core * ACORE:(core + 1) * ACORE]     # [ACORE, NLT, 128, LT]
        gp = np.ascontiguousarray(
            blk.transpose(1, 2, 0, 3).reshape(NLT, 128, ACORE * LT))
        gpf = gp.astype(np.float16)
        gpb = gp.astype(bf)
        in_maps.append({
            **rp,
            "gf0": np.ascontiguousarray(gpf[:4]),
            "gf1": np.ascontiguousarray(gpf[4:]),
            "gb0": np.ascontiguousarray(gpb[:4]),
            "gb1": np.ascontiguousarray(gpb[4:]),
        })
    return in_maps, order, s_t


def kernel(lx, task_matrix, task_difficulty, alg_efficiency, alg_memory,
           alg_experience_boost):
    from concourse.bass_utils import run_bass_kernel_spmd

    in_maps, order, s_t = _host_chain(
        lx, task_matrix, task_difficulty, alg_efficiency, alg_memory,
        alg_experience_boost)

    if "nc" not in _CACHE:
        _CACHE["nc"] = _build_program()
    nc = _CACHE["nc"]

    res = run_bass_kernel_spmd(nc, in_maps, core_ids=list(range(NCORES)),
                               trace=False)
    srt = np.empty((A, T, L), dtype=np.float32)   # sorted-task sig field
    for cidx in range(NCORES):
        d8 = res.results[cidx]["out8"]            # [16, 128, 2048] int8
        for idx in range(NG * NTB):
            g, tb = idx // 8, idx % 8
            arr = d8[idx].astype(np.float32)      # [128t, (a,s,ll)]
            if tb < NX:
                sig = arr / 126.5
            else:
                x = arr / s_t[tb * 128:(tb + 1) * 128][:, None]
                sig = np.tanh(x)
            sig = sig.reshape(128, ACORE, 256).transpose(1, 0, 2)
            srt[cidx * ACORE:(cidx + 1) * ACORE,
                tb * 128:(tb + 1) * 128,
                g * 256:(g + 1) * 256] = sig
    out = np.empty((A, T, L + 1), dtype=np.float32)
    out[:, :, 0] = 0.0
    out[:, order, 1:] = srt
    return out
